# revision 53
# baseline (speedup 1.0000x reference)
"""GAT layer (nn_GATLayer) on 8 Trainium2 NeuronCores via Bass/Tile.

Sharding: 8 cores = batch(4) x dst-half(2). Per-destination softmax
normalizes over the src axis, so splitting the dst axis needs no
cross-device communication. Each core computes feat = node_feat @ W for its
batch (el needs all src nodes), its attention slice, and a partial output
(sum over its dst chunk); the host sums the two partials per batch.

Device layout: dst nodes on partitions, src nodes on the free axis, so the
per-dst softmax over src is a free-axis reduction fused into the Exp
activation's accum_out. The mask enters as an additive -1e9 folded into the
broadcast el tensor (el_b' = el_b - 1e9; logits = adj*1e9 + el_b' + er).
The attention slice is written to DRAM in [dst, src] orientation (dense
[128, 2048] tiles straight from compute layout); the host transposes to the
reference [src, dst, head] layout during gather. Output aggregation runs as
out^T[(h,e), src] = feat_tile^T @ attn, accumulated across dst tiles
entirely in PSUM (two persistent [128, 2048] banks-tiles, heads packed in
pairs via PE tile_position), then transposed on-chip in a short epilogue.

The node order is permuted per-core so the core's dst chunk is always nodes
[0:1024] (keeps the SPMD program identical across cores); the host
un-permutes on assembly.
"""
import sys

if '/opt/trn_rl_repo' not in sys.path:
    sys.path.insert(0, '/opt/trn_rl_repo')

import numpy as np
from contextlib import ExitStack

import concourse.bass as bass
import concourse.bacc as bacc
import concourse.tile as tile
from concourse import mybir
from concourse.masks import make_identity
from concourse.bass_utils import run_bass_kernel_spmd

dt = mybir.dt
AF = mybir.ActivationFunctionType
ALU = mybir.AluOpType

B, N, IN_DIM, H, D = 4, 2048, 256, 4, 64
ALPHA = 0.2
P = 128
DCHUNK = N // 2          # dst nodes per core
NT = N // P              # 16 src blocks
TT = DCHUNK // P         # 8 dst tiles per core
KT = IN_DIM // P         # 2 contraction tiles
NCORES = 8

_NC_CACHE = {}

_SEL = np.zeros((H, H * P), np.float32)
for _h in range(H):
    _SEL[_h, _h * P:(_h + 1) * P] = 1.0


def _build_nc():
    nc = bacc.Bacc("TRN2", target_bir_lowering=False, debug=False)
    f32 = dt.float32

    nfT = nc.dram_tensor("nfT", [IN_DIM, N], f32, kind="ExternalInput").ap()
    adjT = nc.dram_tensor("adjT", [DCHUNK, N], dt.int8, kind="ExternalInput").ap()
    W = nc.dram_tensor("W", [IN_DIM, IN_DIM], f32, kind="ExternalInput").ap()
    # columns 0:4 = W@Al, columns 32:36 = W@Ar (padding keeps the er rows at
    # PSUM base partition 32, which the engines can address)
    WA = nc.dram_tensor("WA", [IN_DIM, 36], f32, kind="ExternalInput").ap()
    elscr = nc.dram_tensor("elscr", [H, N], f32).ap()  # scratch for el broadcast
    # attention slice in [dst-tile, head, dst128, src] orientation
    att = nc.dram_tensor("att", [TT * H * P, N], f32, kind="ExternalOutput").ap()
    # partial output, transposed: rows = (pair, h%2, e), cols = src
    outpT = nc.dram_tensor("outpT", [2 * P, N], f32, kind="ExternalOutput").ap()

    with tile.TileContext(nc) as tc, ExitStack() as ctx:
        consts = ctx.enter_context(tc.tile_pool(name="consts", bufs=1))

        ident = consts.tile([P, P], f32)
        make_identity(nc, ident[:])

        W_sb = consts.tile([P, KT * IN_DIM], f32)
        WA_sb = consts.tile([P, KT * 36], f32)
        for k in range(KT):
            nc.sync.dma_start(W_sb[:, k * IN_DIM:(k + 1) * IN_DIM], W[k * P:(k + 1) * P, :])
            nc.sync.dma_start(WA_sb[:, k * 36:(k + 1) * 36], WA[k * P:(k + 1) * P, :])

        # separate tiles so main-loop consumers gate on exactly what they need
        feat_t = [consts.tile([P, IN_DIM], dt.float16, tag=f"feat{j}", name=f"feat{j}")
                  for j in range(TT)]
        el_bh = [consts.tile([P, N], f32, tag=f"elb{h}", name=f"elb{h}")
                 for h in range(H)]
        er_sb = consts.tile([P, TT * H], f32)          # er columns per (t, h)

        # ---------- preamble: feat, el, er ----------
        with ExitStack() as pre:
            nfT_pool = pre.enter_context(tc.tile_pool(name="pre_nfT", bufs=1))
            tmp_pool = pre.enter_context(tc.tile_pool(name="pre_tmp", bufs=1))
            ps = pre.enter_context(tc.tile_pool(name="pre_ps", bufs=1, space="PSUM"))

            nfT_sb = nfT_pool.tile([P, KT * N], f32)
            for k in range(KT):
                nc.sync.dma_start(nfT_sb[:, k * N:(k + 1) * N], nfT[k * P:(k + 1) * P, :])

            # elT/erT [H, N] = (W @ A).T @ nfT in one fused matmul set
            elT = tmp_pool.tile([H, N], f32)
            erT = tmp_pool.tile([H, N], f32)
            pe_ = ps.tile([36, N], f32, tag="big")
            for k in range(KT):
                for q in range(N // 512):
                    nc.tensor.matmul(
                        pe_[:, q * 512:(q + 1) * 512],
                        WA_sb[:, k * 36:(k + 1) * 36],
                        nfT_sb[:, k * N + q * 512:k * N + (q + 1) * 512],
                        start=(k == 0), stop=(k == KT - 1))
            nc.vector.tensor_copy(elT[:], pe_[0:H, :])
            nc.scalar.copy(erT[:], pe_[32:36, :])

            # er columns: transpose erT [H, 128] chunks -> [128, H]
            for t in range(TT):
                pt = ps.tile([P, H], f32, tag="pf")
                nc.tensor.transpose(pt[:], erT[:, t * P:(t + 1) * P], ident[:H, :H])
                nc.vector.tensor_copy(er_sb[:, t * H:(t + 1) * H], pt[:])

            # el_b[h] = broadcast of elT row h across partitions, via a DRAM
            # round-trip with a 0-step source AP (frees the PE)
            nc.sync.dma_start(elscr[:, :], elT[:])
            for h in range(H):
                src = bass.AP(elscr.tensor, h * N, [[0, P], [1, N]])
                nc.sync.dma_start(el_bh[h][:], src)

            # feat[j-block] = sum_k nfT[k][:, j].T @ W[k]; only the dst-chunk
            # rows, consumed (as fp16) by the aggregation matmuls
            for j in range(TT):
                pf = ps.tile([P, IN_DIM], f32, tag="pf")
                for k in range(KT):
                    nc.tensor.matmul(
                        pf[:], nfT_sb[:, k * N + j * P:k * N + (j + 1) * P],
                        W_sb[:, k * IN_DIM:(k + 1) * IN_DIM],
                        start=(k == 0), stop=(k == KT - 1))
                if j % 2 == 0:
                    nc.scalar.copy(feat_t[j][:], pf[:])
                else:
                    nc.vector.tensor_copy(feat_t[j][:], pf[:])

        # ---------- main loop over dst tiles ----------
        io_pool = ctx.enter_context(tc.tile_pool(name="io", bufs=3))
        work_pool = ctx.enter_context(tc.tile_pool(name="work", bufs=6))
        small = ctx.enter_context(tc.tile_pool(name="small", bufs=8))
        ep_pool = ctx.enter_context(tc.tile_pool(name="ep", bufs=1))
        ps_agg = ctx.enter_context(tc.tile_pool(name="ps_agg", bufs=1, space="PSUM"))

        # persistent PSUM accumulators: out^T[(h%2)*64+e, src] per h-pair
        pair0 = ps_agg.tile([P, N], dt.float32, tag="pair0")
        pair1 = ps_agg.tile([P, N], dt.float32, tag="pair1")
        pairs = [pair0, pair1]

        for t in range(TT):
            adjt = io_pool.tile([P, N], dt.int8, tag="adj")
            nc.sync.dma_start(adjt[:], adjT[t * P:(t + 1) * P, :])

            for h in range(H):
                er_col = er_sb[:, t * H + h:t * H + h + 1]
                w = work_pool.tile([P, N], dt.float32, tag="w")
                # leaky_relu(el + er), unmasked, in one tile computed in place
                nc.scalar.activation(w[:], el_bh[h][:],
                                     AF.Prelu, bias=er_col, alpha=ALPHA)
                nc.scalar.activation(w[:], w[:], AF.Exp)
                # mask multiply with fused per-dst row sum
                colsum = small.tile([P, 1], dt.float32, tag="cs")
                nc.vector.scalar_tensor_tensor(
                    w[:], w[:], 1.0, adjt[:], ALU.mult, ALU.mult,
                    accum_out=colsum[:])
                rec = small.tile([P, 1], dt.float32, tag="rec")
                nc.vector.reciprocal(rec[:], colsum[:])
                # normalize in place -> attention tile [dst128, src]
                nc.vector.tensor_scalar(w[:], w[:], rec[:, 0:1], None, ALU.mult)

                # attention slice out, [dst, src] orientation (host transposes)
                nc.sync.dma_start(att[(t * H + h) * P:(t * H + h + 1) * P, :], w[:])

                # fp16 copy of the attention tile for the aggregation matmul;
                # it only gates the PE, so half go to the (slow) idle GpSimd
                wb = work_pool.tile([P, N], dt.float16, tag="wb")
                cvt = (t * H + h) % 4
                if cvt in (1, 3):
                    nc.gpsimd.tensor_copy(wb[:], w[:])
                elif cvt == 0:
                    nc.vector.tensor_copy(wb[:], w[:])
                else:
                    nc.scalar.copy(wb[:], w[:])

                # aggregation: out^T[(h,e), s] += feat[d,(h,e)].T @ attn[d, s]
                pair = pairs[h // 2]
                off = (h % 2) * D
                frhs = feat_t[t][:, h * D:(h + 1) * D]
                for q in range(N // 512):
                    nc.tensor.matmul(pair[off:off + D, q * 512:(q + 1) * 512],
                                     frhs, wb[:, q * 512:(q + 1) * 512],
                                     start=(t == 0), stop=(t == TT - 1),
                                     skip_group_check=True)

        # ---------- epilogue: flush out^T (host transposes) ----------
        outpT_sb = ep_pool.tile([P, 2 * N], dt.float32)
        nc.vector.tensor_copy(outpT_sb[:, 0:N], pair0[:])
        nc.scalar.copy(outpT_sb[:, N:2 * N], pair1[:])
        nc.sync.dma_start(outpT[0:P, :], outpT_sb[:, 0:N])
        nc.sync.dma_start(outpT[P:2 * P, :], outpT_sb[:, N:2 * N])

    nc.compile()
    return nc


def _get_nc():
    if "nc" not in _NC_CACHE:
        _NC_CACHE["nc"] = _build_nc()
    return _NC_CACHE["nc"]


def _prep_inputs(node_feat, adj_matrix, W, attn_l, attn_r):
    """Build the 8 per-core input maps (host-side layout prep only)."""
    W = np.ascontiguousarray(W, dtype=np.float32)
    Al = np.zeros((H, D, H), np.float32)
    Ar = np.zeros((H, D, H), np.float32)
    for h in range(H):
        Al[h, :, h] = attn_l[0, h, :]
        Ar[h, :, h] = attn_r[0, h, :]
    WA = np.zeros((IN_DIM, 36), np.float32)
    WA[:, 0:H] = W @ Al.reshape(IN_DIM, H)
    WA[:, 32:32 + H] = W @ Ar.reshape(IN_DIM, H)

    in_maps = []
    for c in range(NCORES):
        b, dh = divmod(c, 2)
        d0 = dh * DCHUNK
        nf = np.asarray(node_feat[b], np.float32)
        if dh == 0:
            nfT = np.ascontiguousarray(nf.T)
            adjT = np.ascontiguousarray(np.asarray(adj_matrix[b])[:, :DCHUNK].T, np.int8)
        else:
            # permuted node order: dst chunk first (roll by DCHUNK)
            nfT = np.ascontiguousarray(np.roll(nf.T, -DCHUNK, axis=1))
            a = np.asarray(adj_matrix[b])[:, d0:].T          # [dst, src]
            adjT = np.ascontiguousarray(np.roll(a, -DCHUNK, axis=1), np.int8)
        in_maps.append({"nfT": nfT, "adjT": adjT, "W": W, "WA": WA})
    return in_maps


def _assemble(results):
    attention = np.empty((B, N, N, H), np.float32)
    output = np.zeros((B, N, H * D), np.float32)
    for c in range(NCORES):
        b, dh = divmod(c, 2)
        d0 = dh * DCHUNK
        r = results[c]
        # att: [t, h, d128, s_perm] -> [s_perm, d, h]
        att4 = r["att"].reshape(TT, H, P, N)
        blk = np.ascontiguousarray(att4.transpose(3, 0, 2, 1)).reshape(N, DCHUNK, H)
        # outpT rows = (pair, h%2, e) -> [s, (h, e)]
        po = np.ascontiguousarray(
            r["outpT"].reshape(H, D, N).transpose(2, 0, 1)).reshape(N, H * D)
        if dh == 1:  # un-permute the src axis (roll back)
            attention[b, DCHUNK:, d0:d0 + DCHUNK, :] = blk[:N - DCHUNK]
            attention[b, :DCHUNK, d0:d0 + DCHUNK, :] = blk[N - DCHUNK:]
            output[b, DCHUNK:] += po[:N - DCHUNK]
            output[b, :DCHUNK] += po[N - DCHUNK:]
        else:
            attention[b, :, d0:d0 + DCHUNK, :] = blk
            output[b] += po
    return output, attention


def kernel(node_feat, adj_matrix, W, attn_l, attn_r):
    nc = _get_nc()
    in_maps = _prep_inputs(node_feat, adj_matrix, W, attn_l, attn_r)
    res = run_bass_kernel_spmd(nc, in_maps, core_ids=list(range(NCORES)))
    return _assemble(res.results)


# revision 54
# speedup vs baseline: 1.3585x; 1.3585x over previous
"""GAT layer (nn_GATLayer) on 8 Trainium2 NeuronCores via Bass/Tile.

Sharding: 8 cores = batch(4) x dst-half(2). Per-destination softmax
normalizes over the src axis, so splitting the dst axis needs no
cross-device communication. Each core computes feat = node_feat @ W for its
batch (el needs all src nodes), its attention slice, and a partial output
(sum over its dst chunk); the host sums the two partials per batch.

Device layout: dst nodes on partitions, src nodes on the free axis, so the
per-dst softmax over src is a free-axis reduction fused into the Exp
activation's accum_out. The mask enters as an additive -1e9 folded into the
broadcast el tensor (el_b' = el_b - 1e9; logits = adj*1e9 + el_b' + er).
The attention slice is written to DRAM in [dst, src] orientation (dense
[128, 2048] tiles straight from compute layout); the host transposes to the
reference [src, dst, head] layout during gather. Output aggregation runs as
out^T[(h,e), src] = feat_tile^T @ attn, accumulated across dst tiles
entirely in PSUM (two persistent [128, 2048] banks-tiles, heads packed in
pairs via PE tile_position), then transposed on-chip in a short epilogue.

The node order is permuted per-core so the core's dst chunk is always nodes
[0:1024] (keeps the SPMD program identical across cores); the host
un-permutes on assembly.
"""
import sys

if '/opt/trn_rl_repo' not in sys.path:
    sys.path.insert(0, '/opt/trn_rl_repo')

import numpy as np
from contextlib import ExitStack

import concourse.bass as bass
import concourse.bacc as bacc
import concourse.tile as tile
from concourse import mybir
from concourse.masks import make_identity
from concourse.bass_utils import run_bass_kernel_spmd

dt = mybir.dt
AF = mybir.ActivationFunctionType
ALU = mybir.AluOpType

B, N, IN_DIM, H, D = 4, 2048, 256, 4, 64
ALPHA = 0.2
P = 128
DCHUNK = N // 2          # dst nodes per core
NT = N // P              # 16 src blocks
TT = DCHUNK // P         # 8 dst tiles per core
KT = IN_DIM // P         # 2 contraction tiles
NCORES = 8

_NC_CACHE = {}

_SEL = np.zeros((H, H * P), np.float32)
for _h in range(H):
    _SEL[_h, _h * P:(_h + 1) * P] = 1.0


def _build_nc():
    nc = bacc.Bacc("TRN2", target_bir_lowering=False, debug=False)
    f32 = dt.float32

    nfT = nc.dram_tensor("nfT", [IN_DIM, N], f32, kind="ExternalInput").ap()
    adjT = nc.dram_tensor("adjT", [DCHUNK, N], dt.int8, kind="ExternalInput").ap()
    W = nc.dram_tensor("W", [IN_DIM, IN_DIM], f32, kind="ExternalInput").ap()
    # columns 0:4 = W@Al, columns 32:36 = W@Ar (padding keeps the er rows at
    # PSUM base partition 32, which the engines can address)
    WA = nc.dram_tensor("WA", [IN_DIM, 36], f32, kind="ExternalInput").ap()
    elscr = nc.dram_tensor("elscr", [H, N], f32).ap()  # scratch for el broadcast
    # attention slice in [dst-tile, head, dst128, src] orientation
    att = nc.dram_tensor("att", [TT * H * P, N], f32, kind="ExternalOutput").ap()
    # partial output, transposed: rows = (pair, h%2, e), cols = src
    outpT = nc.dram_tensor("outpT", [2 * P, N], f32, kind="ExternalOutput").ap()

    with tile.TileContext(nc) as tc, ExitStack() as ctx:
        consts = ctx.enter_context(tc.tile_pool(name="consts", bufs=1))

        ident = consts.tile([P, P], f32)
        make_identity(nc, ident[:])

        W_sb = consts.tile([P, KT * IN_DIM], f32)
        WA_sb = consts.tile([P, KT * 36], f32)
        for k in range(KT):
            nc.sync.dma_start(W_sb[:, k * IN_DIM:(k + 1) * IN_DIM], W[k * P:(k + 1) * P, :])
            nc.sync.dma_start(WA_sb[:, k * 36:(k + 1) * 36], WA[k * P:(k + 1) * P, :])

        # separate tiles so main-loop consumers gate on exactly what they need
        feat_t = [consts.tile([P, IN_DIM], dt.float16, tag=f"feat{j}", name=f"feat{j}")
                  for j in range(TT)]
        el_bh = [consts.tile([P, N], f32, tag=f"elb{h}", name=f"elb{h}")
                 for h in range(H)]
        er_sb = consts.tile([P, TT * H], f32)          # er columns per (t, h)

        # ---------- preamble: feat, el, er ----------
        with ExitStack() as pre:
            nfT_pool = pre.enter_context(tc.tile_pool(name="pre_nfT", bufs=1))
            tmp_pool = pre.enter_context(tc.tile_pool(name="pre_tmp", bufs=1))
            ps = pre.enter_context(tc.tile_pool(name="pre_ps", bufs=1, space="PSUM"))

            nfT_sb = nfT_pool.tile([P, KT * N], f32)
            for k in range(KT):
                nc.sync.dma_start(nfT_sb[:, k * N:(k + 1) * N], nfT[k * P:(k + 1) * P, :])

            # elT/erT [H, N] = (W @ A).T @ nfT in one fused matmul set
            elT = tmp_pool.tile([H, N], f32)
            erT = tmp_pool.tile([H, N], f32)
            pe_ = ps.tile([36, N], f32, tag="big")
            for k in range(KT):
                for q in range(N // 512):
                    nc.tensor.matmul(
                        pe_[:, q * 512:(q + 1) * 512],
                        WA_sb[:, k * 36:(k + 1) * 36],
                        nfT_sb[:, k * N + q * 512:k * N + (q + 1) * 512],
                        start=(k == 0), stop=(k == KT - 1))
            nc.vector.tensor_copy(elT[:], pe_[0:H, :])
            nc.scalar.copy(erT[:], pe_[32:36, :])

            # er columns: transpose erT [H, 128] chunks -> [128, H]
            for t in range(TT):
                pt = ps.tile([P, H], f32, tag="pf")
                nc.tensor.transpose(pt[:], erT[:, t * P:(t + 1) * P], ident[:H, :H])
                nc.vector.tensor_copy(er_sb[:, t * H:(t + 1) * H], pt[:])

            # el_b[h] = broadcast of elT row h across partitions, via a DRAM
            # round-trip with a 0-step source AP (frees the PE)
            nc.sync.dma_start(elscr[:, :], elT[:])
            for h in range(H):
                src = bass.AP(elscr.tensor, h * N, [[0, P], [1, N]])
                nc.sync.dma_start(el_bh[h][:], src)

            # feat[j-block] = sum_k nfT[k][:, j].T @ W[k]; only the dst-chunk
            # rows, consumed (as fp16) by the aggregation matmuls
            for j in range(TT):
                pf = ps.tile([P, IN_DIM], f32, tag="pf")
                for k in range(KT):
                    nc.tensor.matmul(
                        pf[:], nfT_sb[:, k * N + j * P:k * N + (j + 1) * P],
                        W_sb[:, k * IN_DIM:(k + 1) * IN_DIM],
                        start=(k == 0), stop=(k == KT - 1))
                if j % 2 == 0:
                    nc.scalar.copy(feat_t[j][:], pf[:])
                else:
                    nc.vector.tensor_copy(feat_t[j][:], pf[:])

        # ---------- main loop over dst tiles ----------
        io_pool = ctx.enter_context(tc.tile_pool(name="io", bufs=3))
        work_pool = ctx.enter_context(tc.tile_pool(name="work", bufs=6))
        small = ctx.enter_context(tc.tile_pool(name="small", bufs=8))
        ep_pool = ctx.enter_context(tc.tile_pool(name="ep", bufs=1))
        ps_agg = ctx.enter_context(tc.tile_pool(name="ps_agg", bufs=1, space="PSUM"))

        # persistent PSUM accumulators: out^T[(h%2)*64+e, src] per h-pair
        pair0 = ps_agg.tile([P, N], dt.float32, tag="pair0")
        pair1 = ps_agg.tile([P, N], dt.float32, tag="pair1")
        pairs = [pair0, pair1]

        for t in range(TT):
            adjt = io_pool.tile([P, N], dt.int8, tag="adj")
            nc.sync.dma_start(adjt[:], adjT[t * P:(t + 1) * P, :])

            for h in range(H):
                er_col = er_sb[:, t * H + h:t * H + h + 1]
                w = work_pool.tile([P, N], dt.float32, tag="w")
                # leaky_relu(el + er), unmasked, in one tile computed in place
                nc.scalar.activation(w[:], el_bh[h][:],
                                     AF.Prelu, bias=er_col, alpha=ALPHA)
                nc.scalar.activation(w[:], w[:], AF.Exp)
                # mask multiply with fused per-dst row sum
                colsum = small.tile([P, 1], dt.float32, tag="cs")
                nc.vector.scalar_tensor_tensor(
                    w[:], w[:], 1.0, adjt[:], ALU.mult, ALU.mult,
                    accum_out=colsum[:])
                rec = small.tile([P, 1], dt.float32, tag="rec")
                nc.vector.reciprocal(rec[:], colsum[:])
                # normalize in place -> attention tile [dst128, src]
                nc.vector.tensor_scalar(w[:], w[:], rec[:, 0:1], None, ALU.mult)

                # attention slice out, [dst, src] orientation (host transposes)
                nc.sync.dma_start(att[(t * H + h) * P:(t * H + h + 1) * P, :], w[:])

                # fp16 copy of the attention tile for the aggregation matmul
                wb = work_pool.tile([P, N], dt.float16, tag="wb")
                if (t * H + h) % 2 == 0:
                    nc.vector.tensor_copy(wb[:], w[:])
                else:
                    nc.scalar.copy(wb[:], w[:])

                # aggregation: out^T[(h,e), s] += feat[d,(h,e)].T @ attn[d, s]
                pair = pairs[h // 2]
                off = (h % 2) * D
                frhs = feat_t[t][:, h * D:(h + 1) * D]
                for q in range(N // 512):
                    nc.tensor.matmul(pair[off:off + D, q * 512:(q + 1) * 512],
                                     frhs, wb[:, q * 512:(q + 1) * 512],
                                     start=(t == 0), stop=(t == TT - 1),
                                     skip_group_check=True)

        # ---------- epilogue: flush out^T (host transposes) ----------
        outpT_sb = ep_pool.tile([P, 2 * N], dt.float32)
        nc.vector.tensor_copy(outpT_sb[:, 0:N], pair0[:])
        nc.scalar.copy(outpT_sb[:, N:2 * N], pair1[:])
        nc.sync.dma_start(outpT[0:P, :], outpT_sb[:, 0:N])
        nc.sync.dma_start(outpT[P:2 * P, :], outpT_sb[:, N:2 * N])

    nc.compile()
    return nc


def _get_nc():
    if "nc" not in _NC_CACHE:
        _NC_CACHE["nc"] = _build_nc()
    return _NC_CACHE["nc"]


def _prep_inputs(node_feat, adj_matrix, W, attn_l, attn_r):
    """Build the 8 per-core input maps (host-side layout prep only)."""
    W = np.ascontiguousarray(W, dtype=np.float32)
    Al = np.zeros((H, D, H), np.float32)
    Ar = np.zeros((H, D, H), np.float32)
    for h in range(H):
        Al[h, :, h] = attn_l[0, h, :]
        Ar[h, :, h] = attn_r[0, h, :]
    WA = np.zeros((IN_DIM, 36), np.float32)
    WA[:, 0:H] = W @ Al.reshape(IN_DIM, H)
    WA[:, 32:32 + H] = W @ Ar.reshape(IN_DIM, H)

    in_maps = []
    for c in range(NCORES):
        b, dh = divmod(c, 2)
        d0 = dh * DCHUNK
        nf = np.asarray(node_feat[b], np.float32)
        if dh == 0:
            nfT = np.ascontiguousarray(nf.T)
            adjT = np.ascontiguousarray(np.asarray(adj_matrix[b])[:, :DCHUNK].T, np.int8)
        else:
            # permuted node order: dst chunk first (roll by DCHUNK)
            nfT = np.ascontiguousarray(np.roll(nf.T, -DCHUNK, axis=1))
            a = np.asarray(adj_matrix[b])[:, d0:].T          # [dst, src]
            adjT = np.ascontiguousarray(np.roll(a, -DCHUNK, axis=1), np.int8)
        in_maps.append({"nfT": nfT, "adjT": adjT, "W": W, "WA": WA})
    return in_maps


def _assemble(results):
    attention = np.empty((B, N, N, H), np.float32)
    output = np.zeros((B, N, H * D), np.float32)
    for c in range(NCORES):
        b, dh = divmod(c, 2)
        d0 = dh * DCHUNK
        r = results[c]
        # att: [t, h, d128, s_perm] -> [s_perm, d, h]
        att4 = r["att"].reshape(TT, H, P, N)
        blk = np.ascontiguousarray(att4.transpose(3, 0, 2, 1)).reshape(N, DCHUNK, H)
        # outpT rows = (pair, h%2, e) -> [s, (h, e)]
        po = np.ascontiguousarray(
            r["outpT"].reshape(H, D, N).transpose(2, 0, 1)).reshape(N, H * D)
        if dh == 1:  # un-permute the src axis (roll back)
            attention[b, DCHUNK:, d0:d0 + DCHUNK, :] = blk[:N - DCHUNK]
            attention[b, :DCHUNK, d0:d0 + DCHUNK, :] = blk[N - DCHUNK:]
            output[b, DCHUNK:] += po[:N - DCHUNK]
            output[b, :DCHUNK] += po[N - DCHUNK:]
        else:
            attention[b, :, d0:d0 + DCHUNK, :] = blk
            output[b] += po
    return output, attention


def kernel(node_feat, adj_matrix, W, attn_l, attn_r):
    nc = _get_nc()
    in_maps = _prep_inputs(node_feat, adj_matrix, W, attn_l, attn_r)
    res = run_bass_kernel_spmd(nc, in_maps, core_ids=list(range(NCORES)))
    return _assemble(res.results)


# revision 57
# speedup vs baseline: 1.3799x; 1.0157x over previous
"""GAT layer (nn_GATLayer) on 8 Trainium2 NeuronCores via Bass/Tile.

Sharding: 8 cores = batch(4) x dst-half(2). Per-destination softmax
normalizes over the src axis, so splitting the dst axis needs no
cross-device communication. Each core computes feat = node_feat @ W for its
batch (el needs all src nodes), its attention slice, and a partial output
(sum over its dst chunk); the host sums the two partials per batch.

Device layout: dst nodes on partitions, src nodes on the free axis, so the
per-dst softmax over src is a free-axis reduction fused into the Exp
activation's accum_out. The mask enters as an additive -1e9 folded into the
broadcast el tensor (el_b' = el_b - 1e9; logits = adj*1e9 + el_b' + er).
The attention slice is written to DRAM in [dst, src] orientation (dense
[128, 2048] tiles straight from compute layout); the host transposes to the
reference [src, dst, head] layout during gather. Output aggregation runs as
out^T[(h,e), src] = feat_tile^T @ attn, accumulated across dst tiles
entirely in PSUM (two persistent [128, 2048] banks-tiles, heads packed in
pairs via PE tile_position), then transposed on-chip in a short epilogue.

The node order is permuted per-core so the core's dst chunk is always nodes
[0:1024] (keeps the SPMD program identical across cores); the host
un-permutes on assembly.
"""
import sys

if '/opt/trn_rl_repo' not in sys.path:
    sys.path.insert(0, '/opt/trn_rl_repo')

import numpy as np
from contextlib import ExitStack

import concourse.bass as bass
import concourse.bacc as bacc
import concourse.tile as tile
from concourse import mybir
from concourse.masks import make_identity
from concourse.bass_utils import run_bass_kernel_spmd

dt = mybir.dt
AF = mybir.ActivationFunctionType
ALU = mybir.AluOpType

B, N, IN_DIM, H, D = 4, 2048, 256, 4, 64
ALPHA = 0.2
P = 128
DCHUNK = N // 2          # dst nodes per core
NT = N // P              # 16 src blocks
TT = DCHUNK // P         # 8 dst tiles per core
KT = IN_DIM // P         # 2 contraction tiles
NCORES = 8

_NC_CACHE = {}

_SEL = np.zeros((H, H * P), np.float32)
for _h in range(H):
    _SEL[_h, _h * P:(_h + 1) * P] = 1.0


def _build_nc():
    nc = bacc.Bacc("TRN2", target_bir_lowering=False, debug=False)
    f32 = dt.float32

    nfT = nc.dram_tensor("nfT", [IN_DIM, N], f32, kind="ExternalInput").ap()
    adjT = nc.dram_tensor("adjT", [DCHUNK, N], dt.int8, kind="ExternalInput").ap()
    W = nc.dram_tensor("W", [IN_DIM, IN_DIM], f32, kind="ExternalInput").ap()
    # columns 0:4 = W@Al, columns 32:36 = W@Ar (padding keeps the er rows at
    # PSUM base partition 32, which the engines can address)
    WA = nc.dram_tensor("WA", [IN_DIM, 36], f32, kind="ExternalInput").ap()
    elscr = nc.dram_tensor("elscr", [H, N], f32).ap()  # scratch for el broadcast
    # attention slice in [dst-tile, head, dst128, src] orientation
    att = nc.dram_tensor("att", [TT * H * P, N], f32, kind="ExternalOutput").ap()
    # partial output, transposed: rows = (pair, h%2, e), cols = src
    outpT = nc.dram_tensor("outpT", [2 * P, N], f32, kind="ExternalOutput").ap()

    with tile.TileContext(nc) as tc, ExitStack() as ctx:
        consts = ctx.enter_context(tc.tile_pool(name="consts", bufs=1))

        ident = consts.tile([P, P], f32)
        make_identity(nc, ident[:])
        shift = consts.tile([P, 1], f32)
        nc.vector.memset(shift[:], -12.0)

        W_sb = consts.tile([P, KT * IN_DIM], f32)
        WA_sb = consts.tile([P, KT * 36], f32)
        for k in range(KT):
            nc.sync.dma_start(W_sb[:, k * IN_DIM:(k + 1) * IN_DIM], W[k * P:(k + 1) * P, :])
            nc.sync.dma_start(WA_sb[:, k * 36:(k + 1) * 36], WA[k * P:(k + 1) * P, :])

        # separate tiles so main-loop consumers gate on exactly what they need
        feat_t = [consts.tile([P, IN_DIM], dt.float16, tag=f"feat{j}", name=f"feat{j}")
                  for j in range(TT)]
        el_bh = [consts.tile([P, N], f32, tag=f"elb{h}", name=f"elb{h}")
                 for h in range(H)]
        er_sb = consts.tile([P, TT * H], f32)          # er columns per (t, h)

        # ---------- preamble: feat, el, er ----------
        with ExitStack() as pre:
            nfT_pool = pre.enter_context(tc.tile_pool(name="pre_nfT", bufs=1))
            tmp_pool = pre.enter_context(tc.tile_pool(name="pre_tmp", bufs=1))
            ps = pre.enter_context(tc.tile_pool(name="pre_ps", bufs=1, space="PSUM"))

            nfT_sb = nfT_pool.tile([P, KT * N], f32)
            for k in range(KT):
                nc.sync.dma_start(nfT_sb[:, k * N:(k + 1) * N], nfT[k * P:(k + 1) * P, :])

            # elT/erT [H, N] = (W @ A).T @ nfT in one fused matmul set
            elT = tmp_pool.tile([H, N], f32)
            erT = tmp_pool.tile([H, N], f32)
            pe_ = ps.tile([36, N], f32, tag="big")
            for k in range(KT):
                for q in range(N // 512):
                    nc.tensor.matmul(
                        pe_[:, q * 512:(q + 1) * 512],
                        WA_sb[:, k * 36:(k + 1) * 36],
                        nfT_sb[:, k * N + q * 512:k * N + (q + 1) * 512],
                        start=(k == 0), stop=(k == KT - 1))
            nc.vector.tensor_copy(elT[:], pe_[0:H, :])
            nc.scalar.copy(erT[:], pe_[32:36, :])

            # er columns: transpose erT [H, 128] chunks -> [128, H]
            for t in range(TT):
                pt = ps.tile([P, H], f32, tag="pf")
                nc.tensor.transpose(pt[:], erT[:, t * P:(t + 1) * P], ident[:H, :H])
                nc.vector.tensor_copy(er_sb[:, t * H:(t + 1) * H], pt[:])

            # el_b[h] = broadcast of elT row h across partitions, via a DRAM
            # round-trip with a 0-step source AP (frees the PE)
            nc.sync.dma_start(elscr[:, :], elT[:])
            for h in range(H):
                src = bass.AP(elscr.tensor, h * N, [[0, P], [1, N]])
                nc.sync.dma_start(el_bh[h][:], src)

            # feat[j-block] = sum_k nfT[k][:, j].T @ W[k]; only the dst-chunk
            # rows, consumed (as fp16) by the aggregation matmuls
            for j in range(TT):
                pf = ps.tile([P, IN_DIM], f32, tag="pf")
                for k in range(KT):
                    nc.tensor.matmul(
                        pf[:], nfT_sb[:, k * N + j * P:k * N + (j + 1) * P],
                        W_sb[:, k * IN_DIM:(k + 1) * IN_DIM],
                        start=(k == 0), stop=(k == KT - 1))
                if j % 2 == 0:
                    nc.scalar.copy(feat_t[j][:], pf[:])
                else:
                    nc.vector.tensor_copy(feat_t[j][:], pf[:])

        # ---------- main loop over dst tiles ----------
        io_pool = ctx.enter_context(tc.tile_pool(name="io", bufs=3))
        work_pool = ctx.enter_context(tc.tile_pool(name="work", bufs=6))
        small = ctx.enter_context(tc.tile_pool(name="small", bufs=8))
        ep_pool = ctx.enter_context(tc.tile_pool(name="ep", bufs=1))
        ps_agg = ctx.enter_context(tc.tile_pool(name="ps_agg", bufs=1, space="PSUM"))

        # persistent PSUM accumulators: out^T[(h%2)*64+e, src] per h-pair
        pair0 = ps_agg.tile([P, N], dt.float32, tag="pair0")
        pair1 = ps_agg.tile([P, N], dt.float32, tag="pair1")
        pairs = [pair0, pair1]

        for t in range(TT):
            adjt = io_pool.tile([P, N], dt.int8, tag="adj")
            nc.sync.dma_start(adjt[:], adjT[t * P:(t + 1) * P, :])

            for h in range(H):
                i = t * H + h
                er_col = er_sb[:, i:i + 1]
                w = work_pool.tile([P, N], dt.float32, tag="w")
                # leaky_relu(el + er), unmasked, then exp(.-12): the constant
                # shift cancels in the softmax and keeps exp in fp16 range
                nc.scalar.activation(w[:], el_bh[h][:],
                                     AF.Prelu, bias=er_col, alpha=ALPHA)
                nc.scalar.activation(w[:], w[:], AF.Exp, bias=shift[:, 0:1])
                colsum = small.tile([P, 1], dt.float32, tag="cs")
                rec = small.tile([P, 1], dt.float32, tag="rec")
                pair = pairs[h // 2]
                off = (h % 2) * D
                if i % 3 == 0:
                    # scheme a: masked sum, normalize, fp16 convert on ACT
                    nc.vector.scalar_tensor_tensor(
                        w[:], w[:], 1.0, adjt[:], ALU.mult, ALU.mult,
                        accum_out=colsum[:])
                    nc.vector.reciprocal(rec[:], colsum[:])
                    nc.vector.tensor_scalar(w[:], w[:], rec[:, 0:1], None, ALU.mult)
                    nc.sync.dma_start(att[i * P:(i + 1) * P, :], w[:])
                    wb = work_pool.tile([P, N], dt.float16, tag="wb")
                    nc.scalar.copy(wb[:], w[:])
                    frhs = feat_t[t][:, h * D:(h + 1) * D]
                else:
                    # scheme c: masked sum straight to fp16 (for the PE);
                    # normalization folded into the feat tile and into the
                    # f32 attention STT
                    wb = work_pool.tile([P, N], dt.float16, tag="wb")
                    nc.vector.scalar_tensor_tensor(
                        wb[:], w[:], 1.0, adjt[:], ALU.mult, ALU.mult,
                        accum_out=colsum[:])
                    nc.vector.reciprocal(rec[:], colsum[:])
                    nc.vector.scalar_tensor_tensor(
                        w[:], w[:], rec[:, 0:1], adjt[:], ALU.mult, ALU.mult)
                    nc.sync.dma_start(att[i * P:(i + 1) * P, :], w[:])
                    fsc = small.tile([P, D], dt.float16, tag="fsc")
                    nc.vector.tensor_scalar(fsc[:], feat_t[t][:, h * D:(h + 1) * D],
                                            rec[:, 0:1], None, ALU.mult)
                    frhs = fsc[:]

                # aggregation: out^T[(h,e), s] += feat[d,(h,e)].T @ attn[d, s]
                for q in range(N // 512):
                    nc.tensor.matmul(pair[off:off + D, q * 512:(q + 1) * 512],
                                     frhs, wb[:, q * 512:(q + 1) * 512],
                                     start=(t == 0), stop=(t == TT - 1),
                                     skip_group_check=True)

        # ---------- epilogue: flush out^T (host transposes) ----------
        outpT_sb = ep_pool.tile([P, 2 * N], dt.float32)
        nc.vector.tensor_copy(outpT_sb[:, 0:N], pair0[:])
        nc.scalar.copy(outpT_sb[:, N:2 * N], pair1[:])
        nc.sync.dma_start(outpT[0:P, :], outpT_sb[:, 0:N])
        nc.sync.dma_start(outpT[P:2 * P, :], outpT_sb[:, N:2 * N])

    nc.compile()
    return nc


def _get_nc():
    if "nc" not in _NC_CACHE:
        _NC_CACHE["nc"] = _build_nc()
    return _NC_CACHE["nc"]


def _prep_inputs(node_feat, adj_matrix, W, attn_l, attn_r):
    """Build the 8 per-core input maps (host-side layout prep only)."""
    W = np.ascontiguousarray(W, dtype=np.float32)
    Al = np.zeros((H, D, H), np.float32)
    Ar = np.zeros((H, D, H), np.float32)
    for h in range(H):
        Al[h, :, h] = attn_l[0, h, :]
        Ar[h, :, h] = attn_r[0, h, :]
    WA = np.zeros((IN_DIM, 36), np.float32)
    WA[:, 0:H] = W @ Al.reshape(IN_DIM, H)
    WA[:, 32:32 + H] = W @ Ar.reshape(IN_DIM, H)

    in_maps = []
    for c in range(NCORES):
        b, dh = divmod(c, 2)
        d0 = dh * DCHUNK
        nf = np.asarray(node_feat[b], np.float32)
        if dh == 0:
            nfT = np.ascontiguousarray(nf.T)
            adjT = np.ascontiguousarray(np.asarray(adj_matrix[b])[:, :DCHUNK].T, np.int8)
        else:
            # permuted node order: dst chunk first (roll by DCHUNK)
            nfT = np.ascontiguousarray(np.roll(nf.T, -DCHUNK, axis=1))
            a = np.asarray(adj_matrix[b])[:, d0:].T          # [dst, src]
            adjT = np.ascontiguousarray(np.roll(a, -DCHUNK, axis=1), np.int8)
        in_maps.append({"nfT": nfT, "adjT": adjT, "W": W, "WA": WA})
    return in_maps


def _assemble(results):
    attention = np.empty((B, N, N, H), np.float32)
    output = np.zeros((B, N, H * D), np.float32)
    for c in range(NCORES):
        b, dh = divmod(c, 2)
        d0 = dh * DCHUNK
        r = results[c]
        # att: [t, h, d128, s_perm] -> [s_perm, d, h]
        att4 = r["att"].reshape(TT, H, P, N)
        blk = np.ascontiguousarray(att4.transpose(3, 0, 2, 1)).reshape(N, DCHUNK, H)
        # outpT rows = (pair, h%2, e) -> [s, (h, e)]
        po = np.ascontiguousarray(
            r["outpT"].reshape(H, D, N).transpose(2, 0, 1)).reshape(N, H * D)
        if dh == 1:  # un-permute the src axis (roll back)
            attention[b, DCHUNK:, d0:d0 + DCHUNK, :] = blk[:N - DCHUNK]
            attention[b, :DCHUNK, d0:d0 + DCHUNK, :] = blk[N - DCHUNK:]
            output[b, DCHUNK:] += po[:N - DCHUNK]
            output[b, :DCHUNK] += po[N - DCHUNK:]
        else:
            attention[b, :, d0:d0 + DCHUNK, :] = blk
            output[b] += po
    return output, attention


def kernel(node_feat, adj_matrix, W, attn_l, attn_r):
    nc = _get_nc()
    in_maps = _prep_inputs(node_feat, adj_matrix, W, attn_l, attn_r)
    res = run_bass_kernel_spmd(nc, in_maps, core_ids=list(range(NCORES)))
    return _assemble(res.results)


# revision 60
# speedup vs baseline: 1.4609x; 1.0587x over previous
"""GAT layer (nn_GATLayer) on 8 Trainium2 NeuronCores via Bass/Tile.

Sharding: 8 cores = batch(4) x dst-half(2). Per-destination softmax
normalizes over the src axis, so splitting the dst axis needs no
cross-device communication. Each core computes feat = node_feat @ W for its
batch (el needs all src nodes), its attention slice, and a partial output
(sum over its dst chunk); the host sums the two partials per batch.

Device layout: dst nodes on partitions, src nodes on the free axis. Per
(dst-tile, head): exp(leaky_relu(el_bcast + er) - 12) on the Scalar engine
(er via the per-partition activation bias; the -12 shift cancels in the
softmax and keeps the unnormalized weights in fp16 range), then one fused
vector op multiplies by the 0/1 mask while accumulating the per-dst row sum
(softmax denominator). The attention slice is written to DRAM in
[dst, src] orientation (dense [128, 2048] tiles straight from the compute
layout); the host transposes to the reference [src, dst, head] layout
during gather. Output aggregation runs in fp16 as out^T[(h,e), src] =
feat_tile^T @ attn, accumulated across dst tiles entirely in PSUM (two
persistent [128, 2048] accumulators, heads packed in pairs via PE
tile_position), and is flushed transposed; the host restores [src, (h,e)].

The node order is permuted per-core so the core's dst chunk is always nodes
[0:1024] (keeps the SPMD program identical across cores); the host
un-permutes on assembly.
"""
import sys

if '/opt/trn_rl_repo' not in sys.path:
    sys.path.insert(0, '/opt/trn_rl_repo')

import numpy as np
from contextlib import ExitStack

import concourse.bass as bass
import concourse.bacc as bacc
import concourse.tile as tile
from concourse import mybir
from concourse.masks import make_identity
from concourse.bass_utils import run_bass_kernel_spmd

dt = mybir.dt
AF = mybir.ActivationFunctionType
ALU = mybir.AluOpType

B, N, IN_DIM, H, D = 4, 2048, 256, 4, 64
ALPHA = 0.2
P = 128
DCHUNK = N // 2          # dst nodes per core
NT = N // P              # 16 src blocks
TT = DCHUNK // P         # 8 dst tiles per core
KT = IN_DIM // P         # 2 contraction tiles
NCORES = 8

_NC_CACHE = {}


def _build_nc():
    nc = bacc.Bacc("TRN2", target_bir_lowering=False, debug=False)
    f32 = dt.float32

    nfT = nc.dram_tensor("nfT", [IN_DIM, N], f32, kind="ExternalInput").ap()
    adjT = nc.dram_tensor("adjT", [DCHUNK, N], dt.int8, kind="ExternalInput").ap()
    W = nc.dram_tensor("W", [IN_DIM, IN_DIM], f32, kind="ExternalInput").ap()
    # columns 0:4 = W@Al, columns 32:36 = W@Ar (padding keeps the er rows at
    # PSUM base partition 32, which the engines can address)
    WA = nc.dram_tensor("WA", [IN_DIM, 36], f32, kind="ExternalInput").ap()
    elscr = nc.dram_tensor("elscr", [H, N], f32).ap()  # scratch for el broadcast
    # attention slice in [dst-tile, head, dst128, src] orientation
    att = nc.dram_tensor("att", [TT * H * P, N], f32, kind="ExternalOutput").ap()
    # partial output, transposed: rows = (pair, h%2, e), cols = src
    outpT = nc.dram_tensor("outpT", [2 * P, N], f32, kind="ExternalOutput").ap()

    with tile.TileContext(nc) as tc, ExitStack() as ctx:
        consts = ctx.enter_context(tc.tile_pool(name="consts", bufs=1))

        ident = consts.tile([P, P], f32)
        make_identity(nc, ident[:])
        shift = consts.tile([P, 1], f32)
        nc.vector.memset(shift[:], -12.0)

        W_sb = consts.tile([P, KT * IN_DIM], f32)
        WA_sb = consts.tile([P, KT * 36], f32)
        for k in range(KT):
            nc.sync.dma_start(W_sb[:, k * IN_DIM:(k + 1) * IN_DIM], W[k * P:(k + 1) * P, :])
            nc.sync.dma_start(WA_sb[:, k * 36:(k + 1) * 36], WA[k * P:(k + 1) * P, :])

        # separate tiles so main-loop consumers gate on exactly what they need
        feat_t = [consts.tile([P, IN_DIM], dt.float16, tag=f"feat{j}", name=f"feat{j}")
                  for j in range(TT)]
        el_bh = [consts.tile([P, N], f32, tag=f"elb{h}", name=f"elb{h}")
                 for h in range(H)]
        er_sb = consts.tile([P, TT * H], f32)          # er columns per (t, h)

        # ---------- preamble: feat, el, er ----------
        with ExitStack() as pre:
            nfT_pool = pre.enter_context(tc.tile_pool(name="pre_nfT", bufs=1))
            tmp_pool = pre.enter_context(tc.tile_pool(name="pre_tmp", bufs=1))
            ps = pre.enter_context(tc.tile_pool(name="pre_ps", bufs=1, space="PSUM"))

            nfT_sb = nfT_pool.tile([P, KT * N], f32)
            for k in range(KT):
                nc.sync.dma_start(nfT_sb[:, k * N:(k + 1) * N], nfT[k * P:(k + 1) * P, :])

            # elT/erT [H, N] = (W @ A).T @ nfT in one fused matmul set
            elT = tmp_pool.tile([H, N], f32)
            erT = tmp_pool.tile([H, N], f32)
            pe_ = ps.tile([36, N], f32, tag="big")
            for k in range(KT):
                for q in range(N // 512):
                    nc.tensor.matmul(
                        pe_[:, q * 512:(q + 1) * 512],
                        WA_sb[:, k * 36:(k + 1) * 36],
                        nfT_sb[:, k * N + q * 512:k * N + (q + 1) * 512],
                        start=(k == 0), stop=(k == KT - 1))
            nc.vector.tensor_copy(elT[:], pe_[0:H, :])
            nc.scalar.copy(erT[:], pe_[32:36, :])

            # er columns: transpose erT [H, 128] chunks -> [128, H]
            for t in range(TT):
                pt = ps.tile([P, H], f32, tag="pf")
                nc.tensor.transpose(pt[:], erT[:, t * P:(t + 1) * P], ident[:H, :H])
                nc.vector.tensor_copy(er_sb[:, t * H:(t + 1) * H], pt[:])

            # el_b[h] = broadcast of elT row h across partitions, via a DRAM
            # round-trip with a 0-step source AP (frees the PE)
            nc.sync.dma_start(elscr[:, :], elT[:])
            for h in range(H):
                src = bass.AP(elscr.tensor, h * N, [[0, P], [1, N]])
                nc.sync.dma_start(el_bh[h][:], src)

            # feat[j-block] = sum_k nfT[k][:, j].T @ W[k]; only the dst-chunk
            # rows, consumed (as fp16) by the aggregation matmuls
            for j in range(TT):
                pf = ps.tile([P, IN_DIM], f32, tag="pf")
                for k in range(KT):
                    nc.tensor.matmul(
                        pf[:], nfT_sb[:, k * N + j * P:k * N + (j + 1) * P],
                        W_sb[:, k * IN_DIM:(k + 1) * IN_DIM],
                        start=(k == 0), stop=(k == KT - 1))
                if j % 2 == 0:
                    nc.scalar.copy(feat_t[j][:], pf[:])
                else:
                    nc.vector.tensor_copy(feat_t[j][:], pf[:])

        # ---------- main loop over dst tiles ----------
        io_pool = ctx.enter_context(tc.tile_pool(name="io", bufs=8))
        work_pool = ctx.enter_context(tc.tile_pool(name="work", bufs=8))
        small = ctx.enter_context(tc.tile_pool(name="small", bufs=8))
        ep_pool = ctx.enter_context(tc.tile_pool(name="ep", bufs=1))
        ps_agg = ctx.enter_context(tc.tile_pool(name="ps_agg", bufs=1, space="PSUM"))

        # persistent PSUM accumulators: out^T[(h%2)*64+e, src] per h-pair
        pair0 = ps_agg.tile([P, N], dt.float32, tag="pair0")
        pair1 = ps_agg.tile([P, N], dt.float32, tag="pair1")
        pairs = [pair0, pair1]

        for t in range(TT):
            adjt = io_pool.tile([P, N], dt.int8, tag="adj")
            nc.sync.dma_start(adjt[:], adjT[t * P:(t + 1) * P, :])

            for h in range(H):
                i = t * H + h
                er_col = er_sb[:, i:i + 1]
                w = work_pool.tile([P, N], dt.float32, tag="w")
                # leaky_relu(el + er), unmasked, then exp(.-12): the constant
                # shift cancels in the softmax and keeps exp in fp16 range
                nc.scalar.activation(w[:], el_bh[h][:],
                                     AF.Prelu, bias=er_col, alpha=ALPHA)
                nc.scalar.activation(w[:], w[:], AF.Exp, bias=shift[:, 0:1])
                colsum = small.tile([P, 1], dt.float32, tag="cs")
                rec = small.tile([P, 1], dt.float32, tag="rec")
                pair = pairs[h // 2]
                off = (h % 2) * D
                if i % 3 == 0:
                    # scheme a: masked sum, normalize, fp16 convert on ACT
                    nc.vector.scalar_tensor_tensor(
                        w[:], w[:], 1.0, adjt[:], ALU.mult, ALU.mult,
                        accum_out=colsum[:])
                    nc.vector.reciprocal(rec[:], colsum[:])
                    nc.vector.tensor_scalar(w[:], w[:], rec[:, 0:1], None, ALU.mult)
                    nc.sync.dma_start(att[i * P:(i + 1) * P, :], w[:])
                    wb = work_pool.tile([P, N], dt.float16, tag="wb")
                    nc.scalar.copy(wb[:], w[:])
                    frhs = feat_t[t][:, h * D:(h + 1) * D]
                else:
                    # scheme c: masked sum straight to fp16 (for the PE);
                    # normalization folded into the feat tile and into the
                    # f32 attention STT
                    wb = work_pool.tile([P, N], dt.float16, tag="wb")
                    nc.vector.scalar_tensor_tensor(
                        wb[:], w[:], 1.0, adjt[:], ALU.mult, ALU.mult,
                        accum_out=colsum[:])
                    nc.vector.reciprocal(rec[:], colsum[:])
                    nc.vector.scalar_tensor_tensor(
                        w[:], w[:], rec[:, 0:1], adjt[:], ALU.mult, ALU.mult)
                    nc.sync.dma_start(att[i * P:(i + 1) * P, :], w[:])
                    fsc = small.tile([P, D], dt.float16, tag="fsc")
                    nc.vector.tensor_scalar(fsc[:], feat_t[t][:, h * D:(h + 1) * D],
                                            rec[:, 0:1], None, ALU.mult)
                    frhs = fsc[:]

                # aggregation: out^T[(h,e), s] += feat[d,(h,e)].T @ attn[d, s]
                for q in range(N // 512):
                    nc.tensor.matmul(pair[off:off + D, q * 512:(q + 1) * 512],
                                     frhs, wb[:, q * 512:(q + 1) * 512],
                                     start=(t == 0), stop=(t == TT - 1),
                                     skip_group_check=True)

        # ---------- epilogue: flush out^T (host transposes) ----------
        outpT_sb = ep_pool.tile([P, 2 * N], dt.float32)
        nc.vector.tensor_copy(outpT_sb[:, 0:N], pair0[:])
        nc.scalar.copy(outpT_sb[:, N:2 * N], pair1[:])
        nc.sync.dma_start(outpT[0:P, :], outpT_sb[:, 0:N])
        nc.sync.dma_start(outpT[P:2 * P, :], outpT_sb[:, N:2 * N])

    nc.compile()
    return nc


def _get_nc():
    if "nc" not in _NC_CACHE:
        _NC_CACHE["nc"] = _build_nc()
    return _NC_CACHE["nc"]


def _prep_inputs(node_feat, adj_matrix, W, attn_l, attn_r):
    """Build the 8 per-core input maps (host-side layout prep only)."""
    W = np.ascontiguousarray(W, dtype=np.float32)
    Al = np.zeros((H, D, H), np.float32)
    Ar = np.zeros((H, D, H), np.float32)
    for h in range(H):
        Al[h, :, h] = attn_l[0, h, :]
        Ar[h, :, h] = attn_r[0, h, :]
    WA = np.zeros((IN_DIM, 36), np.float32)
    WA[:, 0:H] = W @ Al.reshape(IN_DIM, H)
    WA[:, 32:32 + H] = W @ Ar.reshape(IN_DIM, H)

    in_maps = []
    for c in range(NCORES):
        b, dh = divmod(c, 2)
        d0 = dh * DCHUNK
        nf = np.asarray(node_feat[b], np.float32)
        if dh == 0:
            nfT = np.ascontiguousarray(nf.T)
            adjT = np.ascontiguousarray(np.asarray(adj_matrix[b])[:, :DCHUNK].T, np.int8)
        else:
            # permuted node order: dst chunk first (roll by DCHUNK)
            nfT = np.ascontiguousarray(np.roll(nf.T, -DCHUNK, axis=1))
            a = np.asarray(adj_matrix[b])[:, d0:].T          # [dst, src]
            adjT = np.ascontiguousarray(np.roll(a, -DCHUNK, axis=1), np.int8)
        in_maps.append({"nfT": nfT, "adjT": adjT, "W": W, "WA": WA})
    return in_maps


def _assemble(results):
    attention = np.empty((B, N, N, H), np.float32)
    output = np.zeros((B, N, H * D), np.float32)
    for c in range(NCORES):
        b, dh = divmod(c, 2)
        d0 = dh * DCHUNK
        r = results[c]
        # att: [t, h, d128, s_perm] -> [s_perm, d, h]
        att4 = r["att"].reshape(TT, H, P, N)
        blk = np.ascontiguousarray(att4.transpose(3, 0, 2, 1)).reshape(N, DCHUNK, H)
        # outpT rows = (pair, h%2, e) -> [s, (h, e)]
        po = np.ascontiguousarray(
            r["outpT"].reshape(H, D, N).transpose(2, 0, 1)).reshape(N, H * D)
        if dh == 1:  # un-permute the src axis (roll back)
            attention[b, DCHUNK:, d0:d0 + DCHUNK, :] = blk[:N - DCHUNK]
            attention[b, :DCHUNK, d0:d0 + DCHUNK, :] = blk[N - DCHUNK:]
            output[b, DCHUNK:] += po[:N - DCHUNK]
            output[b, :DCHUNK] += po[N - DCHUNK:]
        else:
            attention[b, :, d0:d0 + DCHUNK, :] = blk
            output[b] += po
    return output, attention


def kernel(node_feat, adj_matrix, W, attn_l, attn_r):
    nc = _get_nc()
    in_maps = _prep_inputs(node_feat, adj_matrix, W, attn_l, attn_r)
    res = run_bass_kernel_spmd(nc, in_maps, core_ids=list(range(NCORES)))
    return _assemble(res.results)
